# revision 6
# baseline (speedup 1.0000x reference)
"""AdaConv2d (per-sample masked 3x3 conv) on 8 TRN2 NeuronCores.

Strategy (data-parallel, per sharding hint):
  - 64 samples sharded 8-per-core; kernel_base/kernel_mask replicated.
  - Two samples share one 128-partition SBUF tile: sample A's padded
    image (one input channel per partition) in partitions 0-63, sample
    B's in 64-127. No shifted copies -> input DMA is 1x the image.
  - Each of the 9 conv taps is one K=64 matmul; per (tap, 4-row block)
    four M=64 matmuls run concurrently on the four 64x64 quadrants of
    the PE array (tile_position auto-derived from base partitions):
      (row 0,  col 0)  = sample A, even block -> psA[0:64]
      (row 0,  col 64) = sample A, odd block  -> psA[64:128]
      (row 64, col 0)  = sample B, even block -> psB[0:64]
      (row 64, col 64) = sample B, odd block  -> psB[64:128]
    so all 16384 MACs/cycle are live on every pass (100% MAC
    utilization; the PE roofline for this decomposition is ~94us).
  - Per-sample kernels (kernel_base * kernel_mask[label], bf16, lhsT
    layout) are precomputed on the host.
  - v3 changes (trace-driven; v1 = 117.6us, v2 = 140.5us regression):
    * PSUM evacuation split across engines: psA -> vector tensor_copy,
      psB -> scalar (ACT) copy; each lands in its own bf16 stage tile,
      so the two halves of a round evacuate concurrently and the last
      round's output DMA starts ~1.3us sooner.
    * Only sync (SP) and scalar (ACT) own hardware DGE queues; gpsimd's
      software queue is slow and adds a ~10.8us dge_drain at teardown
      (v2's mistake). All DMA rides sync + scalar: sync issues both
      output halves per round; scalar issues input prefetch.
    * Input chunks capped at 8 rows (~0.23MB) so no transfer holds a
      DMA completion-semaphore slot long (v1 convoyed output DMAs
      behind 0.5-1MB input transfers in that shared rotation, stalling
      the stage-tile -> CAST -> PSUM-recycle chain, LDW stalls to
      2.7us), and later pairs' chunks are prefetched one-per-round
      inside the previous pair's loop.
    * Pair-0 weights land per-tap (9 small sync DMAs) so the first
      LDWEIGHTS waits on a 33KB slice, not the 295KB tile.
    * PE warmup (HAM clock-gate 1.2->2.4 GHz needs ~3.4us of activity)
      memsets its dummy tile on the vector queue, which clears its
      preamble earliest; warmup matmuls start ~1.3us sooner.
"""
import numpy as np
import ml_dtypes

import concourse.bass as bass  # noqa: F401  (registers engines)
import concourse.tile as tile
from concourse import bacc, mybir
from concourse.bass_utils import run_bass_kernel_spmd

NCORES = 8
SPC = 8            # samples per core
PAIRS = SPC // 2   # two samples share one 128-partition tile
H = W = 112
IC = OC = 64
ND = 4             # demographic groups
PW = H + 2         # padded width/height
PHW = PW * PW
RB = 4             # output rows per matmul block
N = RB * W         # 448 columns per matmul (one PSUM bank)
ROUNDS = H // (2 * RB)   # 14 rounds of (even, odd) blocks per sample
NTAP = 9
FUSE_EPOCH = 9
F32 = mybir.dt.float32
BF16 = mybir.dt.bfloat16

# pair-0 x chunk boundaries (padded-row units): fine leading chunks so
# round 0 starts ASAP; all on the scalar queue ahead of its steady
# work (the first CAST-B isn't needed until ~4 rounds in)
XCH0 = [0, 6, 12, 20, 28, 36, 44, 52, 60, 68, 76, 84, 92, 100, 108, PW]
# pairs >=1: 14 chunks of <=8 rows, issued one per round of the
# previous pair (short transfers keep the shared DMA completion-sem
# rotation moving)
XCHN = [0, 8, 16, 24, 32, 40, 48, 56, 64, 72, 80, 88, 96, 104, PW]
NWARM = 32         # dummy matmuls to lift the PE HAM clock-gate early

_CACHE = {}


def _build():
    nc = bacc.Bacc("TRN2", target_bir_lowering=False, debug=False,
                   num_devices=NCORES)
    xs = nc.dram_tensor("xs", [PAIRS, 128, PHW], BF16,
                        kind="ExternalInput").ap()
    wd = nc.dram_tensor("wd", [PAIRS, 128, NTAP * 128], BF16,
                        kind="ExternalInput").ap()
    out = nc.dram_tensor("out", [PAIRS, ROUNDS, 2, 2, OC, N], BF16,
                         kind="ExternalOutput").ap()

    # [pair, round, sample-in-pair, blk*oc (partition), rb*w]
    ov = out.rearrange("pr r b k oc f -> pr r b (k oc) f")
    wdr = wd.rearrange("pr p (j m) -> pr p j m", m=128)

    with tile.TileContext(nc) as tc:
        with (
            tc.tile_pool(name="xp", bufs=3) as xp,
            tc.tile_pool(name="wp", bufs=2) as wp,
            tc.tile_pool(name="sta", bufs=6) as sap,
            tc.tile_pool(name="stb", bufs=6) as sbp,
            tc.tile_pool(name="psum", bufs=4, space="PSUM") as pp,
            tc.tile_pool(name="warm", bufs=1) as wmp,
        ):
            # warm up the PE HAM clock-gate with dummy matmuls while the
            # first pair's inputs are in flight; memset on the vector
            # queue (earliest past its preamble). The scratch PSUM
            # borrows a generation of the main pool.
            warm = wmp.tile([128, 128], BF16, name="warm", tag="warm")
            nc.vector.memset(warm[:], 0)
            psW = pp.tile([128, N], F32, name="psW", tag="psA")
            for _ in range(NWARM):
                nc.tensor.matmul(psW[0:64, 0:128], warm[:, 0:64], warm[:],
                                 start=True, stop=True)

            xts, wts = {}, {}

            def xt_for(p):
                if p not in xts:
                    t = xp.tile([128, PHW], BF16, name="xt", tag="xt")
                    xts[p] = (t, t.rearrange("p (r c) -> p r c", c=PW))
                return xts[p]

            def wt_for(p):
                if p not in wts:
                    t = wp.tile([128, NTAP * 128], BF16, name="wt",
                                tag="wt")
                    wts[p] = (t, t.rearrange("p (j m) -> p j m", m=128))
                return wts[p]

            # pair 0 inputs: per-tap weight slices on sync (first LDW
            # only waits 33KB), x head chunks on scalar
            _, w30 = wt_for(0)
            for j in range(NTAP):
                nc.sync.dma_start(w30[:, j, :], wdr[0, :, j, :])
            xt0, _ = xt_for(0)
            for a, b in zip(XCH0, XCH0[1:]):
                nc.scalar.dma_start(xt0[:, a * PW:b * PW],
                                    xs[0][:, a * PW:b * PW])

            for pr in range(PAIRS):
                _, x3 = xt_for(pr)
                _, w3 = wt_for(pr)

                for rnd in range(ROUNDS):
                    psA = pp.tile([128, N], F32, name="psA", tag="psA")
                    psB = pp.tile([128, N], F32, name="psB", tag="psB")
                    for j in range(NTAP):
                        dy, dx = divmod(j, 3)
                        first, last = (j == 0), (j == NTAP - 1)
                        for blk in range(2):
                            r0 = rnd * 2 * RB + blk * RB + dy
                            pc = blk * 64
                            rA = x3[0:64, r0:r0 + RB, dx:dx + W]
                            rB = x3[64:128, r0:r0 + RB, dx:dx + W]
                            nc.tensor.matmul(psA[pc:pc + 64, :],
                                             w3[0:64, j, pc:pc + 64], rA,
                                             start=first, stop=last)
                            nc.tensor.matmul(psB[pc:pc + 64, :],
                                             w3[64:128, j, pc:pc + 64], rB,
                                             start=first, stop=last)

                    stA = sap.tile([128, N], BF16, name="stA", tag="stA")
                    stB = sbp.tile([128, N], BF16, name="stB", tag="stB")
                    nc.vector.tensor_copy(stA[:], psA[:])
                    nc.scalar.copy(stB[:], psB[:])
                    nc.sync.dma_start(ov[pr, rnd, 0], stA[:])
                    nc.sync.dma_start(ov[pr, rnd, 1], stB[:])

                    # prefetch next pair: weights in round 0 (sync),
                    # one x chunk per round (scalar issue, ~0.6us slack)
                    if pr + 1 < PAIRS:
                        if rnd == 0:
                            wtn, _ = wt_for(pr + 1)
                            nc.sync.dma_start(wtn[:], wd[pr + 1])
                            xtn, _ = xt_for(pr + 1)
                            a, b = XCHN[0], XCHN[1]
                            nc.scalar.dma_start(xtn[:, a * PW:b * PW],
                                                xs[pr + 1][:, a * PW:b * PW])
                        elif rnd < len(XCHN) - 1:
                            xtn, _ = xt_for(pr + 1)
                            a, b = XCHN[rnd], XCHN[rnd + 1]
                            nc.scalar.dma_start(xtn[:, a * PW:b * PW],
                                                xs[pr + 1][:, a * PW:b * PW])

    nc.compile()
    return nc


def get_nc():
    if "nc" not in _CACHE:
        _CACHE["nc"] = _build()
    return _CACHE["nc"]


def make_in_maps(x, kernel_base, kernel_mask, demog_label, epoch):
    kb = np.asarray(kernel_base, dtype=np.float32)
    km = np.asarray(kernel_mask, dtype=np.float32)
    labels = np.asarray(demog_label).astype(np.int64)
    if int(np.asarray(epoch)) >= FUSE_EPOCH:
        labels = np.zeros_like(labels)

    B = labels.shape[0]
    # padded bf16 image per sample (layout only); pairs share a tile
    xb = np.asarray(x, dtype=np.float32).astype(ml_dtypes.bfloat16)
    xpad = np.zeros((B, IC, PW, PW), dtype=ml_dtypes.bfloat16)
    xpad[:, :, 1:H + 1, 1:W + 1] = xb
    xfull = xpad.reshape(B // 2, 128, PHW)

    # per-sample lhsT weights [ic, tap, oc], duplicated across the two
    # 64-col halves of the PE array
    kbT = kb.reshape(OC, IC, NTAP).transpose(1, 2, 0)   # [ic, j, oc]
    km9 = km.reshape(ND, IC, NTAP)                      # [d, ic, j]
    # ws[d, ic, j, oc] = kb[oc, ic, j] * km[d, ic, j]
    ws = kbT[None] * km9[:, :, :, None]                 # [d, ic, j, oc]
    wdup = np.concatenate([ws, ws], axis=3)             # [d, ic, j, 128]
    wdup = wdup.reshape(ND, IC, NTAP * 128).astype(ml_dtypes.bfloat16)

    in_maps = []
    for c in range(NCORES):
        lab = labels[c * SPC:(c + 1) * SPC]
        wdc = np.zeros((PAIRS, 128, NTAP * 128), dtype=ml_dtypes.bfloat16)
        for p in range(PAIRS):
            wdc[p, 0:IC] = wdup[lab[2 * p]]
            wdc[p, IC:] = wdup[lab[2 * p + 1]]
        in_maps.append({
            "xs": np.ascontiguousarray(
                xfull[c * PAIRS:(c + 1) * PAIRS]),
            "wd": wdc,
        })
    return in_maps


def kernel(x, kernel_base, kernel_mask, demog_label, epoch):
    nc = get_nc()
    in_maps = make_in_maps(x, kernel_base, kernel_mask, demog_label, epoch)
    res = run_bass_kernel_spmd(nc, in_maps, list(range(NCORES)))
    outs = []
    for c in range(NCORES):
        raw = res.results[c]["out"].astype(np.float32)
        # [PAIRS, ROUNDS, b, blk, OC, RB, W] -> [PAIRS, b, OC, R, blk, RB, W]
        raw = raw.reshape(PAIRS, ROUNDS, 2, 2, OC, RB, W)
        raw = raw.transpose(0, 2, 4, 1, 3, 5, 6)
        outs.append(raw.reshape(SPC, OC, H, W))
    return np.concatenate(outs, axis=0)


# revision 7
# speedup vs baseline: 1.0216x; 1.0216x over previous
"""AdaConv2d (per-sample masked 3x3 conv) on 8 TRN2 NeuronCores.

Strategy (data-parallel, per sharding hint):
  - 64 samples sharded 8-per-core; kernel_base/kernel_mask replicated.
  - Two samples share one 128-partition SBUF tile: sample A's padded
    image (one input channel per partition) in partitions 0-63, sample
    B's in 64-127. No shifted copies -> input DMA is 1x the image.
  - Each of the 9 conv taps is one K=64 matmul; per (tap, 4-row block)
    four M=64 matmuls run concurrently on the four 64x64 quadrants of
    the PE array (tile_position auto-derived from base partitions), so
    all 16384 MACs/cycle are live on every pass (PE roofline for this
    decomposition ~94us; measured cadence is at the issue floor).
  - Per-sample kernels (kernel_base * kernel_mask[label], bf16, lhsT
    layout) are precomputed on the host.
  - v4 changes vs v1 (117.6us), trace-driven:
    * Round output staged in two tiles (stA/stB) with two sync-queue
      DMAs per round instead of one combined tile+DMA. v1's single
      0.23MB output DMA convoyed behind 0.5MB input transfers in the
      shared ~8-slot DMA completion-semaphore rotation; the stage-tile
      recycle then stalled CASTs -> PSUM recycle -> LDWEIGHTS (~3us).
      Smaller transfer units keep the rotation moving, and the final
      round's DMA tail shrinks ~1.5us.
    * All input x chunks capped at 8 rows (~0.23MB, ~0.9us on the
      wire) for the same reason.
    * Both CASTs stay on the vector queue: the scalar/gpsimd variants
      regressed badly - scalar's CASTs got stuck behind input-issue
      bursts (queue coupling), gpsimd's DMA queue is software-DGE with
      a ~10.8us teardown drain. Scalar keeps only dependency-free
      input issues; gpsimd does nothing.
    * PE warmup memset rides the vector queue (clears its preamble
      ~1us before gpsimd), and pair 0's first x chunk is 8 rows.
"""
import numpy as np
import ml_dtypes

import concourse.bass as bass  # noqa: F401  (registers engines)
import concourse.tile as tile
from concourse import bacc, mybir
from concourse.bass_utils import run_bass_kernel_spmd

NCORES = 8
SPC = 8            # samples per core
PAIRS = SPC // 2   # two samples share one 128-partition tile
H = W = 112
IC = OC = 64
ND = 4             # demographic groups
PW = H + 2         # padded width/height
PHW = PW * PW
RB = 4             # output rows per matmul block
N = RB * W         # 448 columns per matmul (one PSUM bank)
ROUNDS = H // (2 * RB)   # 14 rounds of (even, odd) blocks per sample
NTAP = 9
FUSE_EPOCH = 9
F32 = mybir.dt.float32
BF16 = mybir.dt.bfloat16

# x chunk boundaries (padded-row units): <=8 rows per chunk so no
# transfer holds a DMA completion-semaphore slot long
XROWS = list(range(0, PW, 8)) + [PW]
NWARM = 32         # dummy matmuls to lift the PE HAM clock-gate early

_CACHE = {}


def _build():
    nc = bacc.Bacc("TRN2", target_bir_lowering=False, debug=False,
                   num_devices=NCORES)
    xs = nc.dram_tensor("xs", [PAIRS, 128, PHW], BF16,
                        kind="ExternalInput").ap()
    wd = nc.dram_tensor("wd", [PAIRS, 128, NTAP * 128], BF16,
                        kind="ExternalInput").ap()
    out = nc.dram_tensor("out", [PAIRS, ROUNDS, 2, 2, OC, N], BF16,
                         kind="ExternalOutput").ap()

    # [pair, round, sample-in-pair, blk*oc (partition), rb*w]
    ov = out.rearrange("pr r b k oc f -> pr r b (k oc) f")

    with tile.TileContext(nc) as tc:
        with (
            tc.tile_pool(name="xp", bufs=3) as xp,
            tc.tile_pool(name="wp", bufs=2) as wp,
            tc.tile_pool(name="sta", bufs=6) as sap,
            tc.tile_pool(name="stb", bufs=6) as sbp,
            tc.tile_pool(name="psum", bufs=4, space="PSUM") as pp,
            tc.tile_pool(name="warm", bufs=1) as wmp,
        ):
            # warm up the PE HAM clock-gate (1.2 -> 2.4 GHz needs ~3.4us
            # of sustained activity) with dummy matmuls on scratch data
            # while the first pair's inputs are in flight; memset on the
            # vector queue, which clears its preamble earliest. The
            # scratch PSUM borrows a generation of the main pool.
            warm = wmp.tile([128, 128], BF16, name="warm", tag="warm")
            nc.vector.memset(warm[:], 0)
            psW = pp.tile([128, N], F32, name="psW", tag="psA")
            for _ in range(NWARM):
                nc.tensor.matmul(psW[0:64, 0:128], warm[:, 0:64], warm[:],
                                 start=True, stop=True)

            for pr in range(PAIRS):
                wt = wp.tile([128, NTAP * 128], BF16, name="wt", tag="wt")
                # pair 0: sync queue is idle, issue there for earliest
                # landing; later pairs: sync is busy with output DMAs, so
                # scalar (whose x-chunk issues are long done) is sooner
                weng = nc.sync if pr == 0 else nc.scalar
                weng.dma_start(wt[:], wd[pr])
                w3 = wt.rearrange("p (j m) -> p j m", m=128)

                xt = xp.tile([128, PHW], BF16, name="xt", tag="xt")
                x3 = xt.rearrange("p (r c) -> p r c", c=PW)
                for q in range(len(XROWS) - 1):
                    qs, qe = XROWS[q] * PW, XROWS[q + 1] * PW
                    nc.scalar.dma_start(xt[:, qs:qe], xs[pr][:, qs:qe])

                for rnd in range(ROUNDS):
                    psA = pp.tile([128, N], F32, name="psA", tag="psA")
                    psB = pp.tile([128, N], F32, name="psB", tag="psB")
                    for j in range(NTAP):
                        dy, dx = divmod(j, 3)
                        first, last = (j == 0), (j == NTAP - 1)
                        for blk in range(2):
                            r0 = rnd * 2 * RB + blk * RB + dy
                            pc = blk * 64
                            rA = x3[0:64, r0:r0 + RB, dx:dx + W]
                            rB = x3[64:128, r0:r0 + RB, dx:dx + W]
                            nc.tensor.matmul(psA[pc:pc + 64, :],
                                             w3[0:64, j, pc:pc + 64], rA,
                                             start=first, stop=last)
                            nc.tensor.matmul(psB[pc:pc + 64, :],
                                             w3[64:128, j, pc:pc + 64], rB,
                                             start=first, stop=last)

                    stA = sap.tile([128, N], BF16, name="stA", tag="stA")
                    stB = sbp.tile([128, N], BF16, name="stB", tag="stB")
                    nc.vector.tensor_copy(stA[:], psA[:])
                    nc.vector.tensor_copy(stB[:], psB[:])
                    nc.sync.dma_start(ov[pr, rnd, 0], stA[:])
                    nc.sync.dma_start(ov[pr, rnd, 1], stB[:])

    nc.compile()
    return nc


def get_nc():
    if "nc" not in _CACHE:
        _CACHE["nc"] = _build()
    return _CACHE["nc"]


def make_in_maps(x, kernel_base, kernel_mask, demog_label, epoch):
    kb = np.asarray(kernel_base, dtype=np.float32)
    km = np.asarray(kernel_mask, dtype=np.float32)
    labels = np.asarray(demog_label).astype(np.int64)
    if int(np.asarray(epoch)) >= FUSE_EPOCH:
        labels = np.zeros_like(labels)

    B = labels.shape[0]
    # padded bf16 image per sample (layout only); pairs share a tile
    xb = np.asarray(x, dtype=np.float32).astype(ml_dtypes.bfloat16)
    xpad = np.zeros((B, IC, PW, PW), dtype=ml_dtypes.bfloat16)
    xpad[:, :, 1:H + 1, 1:W + 1] = xb
    xfull = xpad.reshape(B // 2, 128, PHW)

    # per-sample lhsT weights [ic, tap, oc], duplicated across the two
    # 64-col halves of the PE array
    kbT = kb.reshape(OC, IC, NTAP).transpose(1, 2, 0)   # [ic, j, oc]
    km9 = km.reshape(ND, IC, NTAP)                      # [d, ic, j]
    # ws[d, ic, j, oc] = kb[oc, ic, j] * km[d, ic, j]
    ws = kbT[None] * km9[:, :, :, None]                 # [d, ic, j, oc]
    wdup = np.concatenate([ws, ws], axis=3)             # [d, ic, j, 128]
    wdup = wdup.reshape(ND, IC, NTAP * 128).astype(ml_dtypes.bfloat16)

    in_maps = []
    for c in range(NCORES):
        lab = labels[c * SPC:(c + 1) * SPC]
        wdc = np.zeros((PAIRS, 128, NTAP * 128), dtype=ml_dtypes.bfloat16)
        for p in range(PAIRS):
            wdc[p, 0:IC] = wdup[lab[2 * p]]
            wdc[p, IC:] = wdup[lab[2 * p + 1]]
        in_maps.append({
            "xs": np.ascontiguousarray(
                xfull[c * PAIRS:(c + 1) * PAIRS]),
            "wd": wdc,
        })
    return in_maps


def kernel(x, kernel_base, kernel_mask, demog_label, epoch):
    nc = get_nc()
    in_maps = make_in_maps(x, kernel_base, kernel_mask, demog_label, epoch)
    res = run_bass_kernel_spmd(nc, in_maps, list(range(NCORES)))
    outs = []
    for c in range(NCORES):
        raw = res.results[c]["out"].astype(np.float32)
        # [PAIRS, ROUNDS, b, blk, OC, RB, W] -> [PAIRS, b, OC, R, blk, RB, W]
        raw = raw.reshape(PAIRS, ROUNDS, 2, 2, OC, RB, W)
        raw = raw.transpose(0, 2, 4, 1, 3, 5, 6)
        outs.append(raw.reshape(SPC, OC, H, W))
    return np.concatenate(outs, axis=0)
